# revision 9
# baseline (speedup 1.0000x reference)
"""Trainium2 Bass kernel: 48-bit barrel shifter (right) with sticky bit.

Problem: X [B,48] of 0/1 f32 bits, shift [B,6] of 0/1 f32 bits (MSB first:
32,16,8,4,2,1). Output = (X >> S) with zero fill, plus sticky = OR of the
bits shifted out (= OR of last min(S,48) bits of X).

Strategy (pure data parallel over 8 NeuronCores, 125k rows/core):
  - Tile rows as [P=128 partitions, G rows/partition]; each row's 48 bits
    live in an 80-wide padded block (32 zeros | 48 data) so a right-shift
    by `a` is a read at offset -a with zeros flowing in from the pad.
  - Each of the 6 shift layers is ONE in-place copy_predicated on the DVE,
    streamed in descending column order (negative-step APs) so the offset
    read always happens before the corresponding write (no ping-pong).
    The per-row mask is the shift bit read through a stride-0 broadcast AP.
  - sticky = (sum(X_row) - sum(out_row)) > 0: bits are only moved or
    dropped, so the dropped-bit count is the difference of the two sums.
"""

import numpy as np
from contextlib import ExitStack

import concourse.bacc as bacc
import concourse.mybir as mybir
from concourse import tile
from concourse.bass_utils import run_bass_kernel_spmd

DATA = 48
NSH = 6
PAD = 32
W = PAD + DATA  # 80
SHIFT_AMTS = [32, 16, 8, 4, 2, 1]

N_CORES = 8
B_FULL = 1_000_000
R_CORE = B_FULL // N_CORES  # 125000

# 125000 = 16 tiles of 128x61 (124928) + one tail tile of 72 rows.
FULL_PLAN = [(128, 61, 16), (72, 1, 1)]

F32 = mybir.dt.float32


def build_barrel(nc, R, plan):
    X = nc.dram_tensor("X", [R, DATA], F32, kind="ExternalInput").ap()
    SH = nc.dram_tensor("shift", [R, NSH], F32, kind="ExternalInput").ap()
    OUT = nc.dram_tensor("out", [R, DATA], F32, kind="ExternalOutput").ap()
    STK = nc.dram_tensor("sticky", [R, 1], F32, kind="ExternalOutput").ap()

    g_max = max(g for (_, g, _) in plan)

    with tile.TileContext(nc) as tc:
        with ExitStack() as ctx:
            iopool = ctx.enter_context(tc.tile_pool(name="io", bufs=3))
            wpool = ctx.enter_context(tc.tile_pool(name="work", bufs=1))

            # Persistent ping-pong work buffers; pad columns stay zero for
            # the whole kernel (blends only ever write the data region).
            wbufs = [
                wpool.tile([128, g_max * W], F32, tag=f"wb{i}", name=f"wb{i}")
                for i in range(2)
            ]
            for b in wbufs:
                nc.gpsimd.memset(b[:], 0.0)

            row0 = 0
            it = 0
            for (P, G, T) in plan:
                for _ in range(T):
                    rows = P * G
                    buf = wbufs[it % 2]
                    it += 1

                    bufw = buf[0:P, 0 : G * W].rearrange("p (g w) -> p g w", w=W)
                    data = bufw[:, :, PAD:W]  # [P, G, 48]

                    xin = X[row0 : row0 + rows, :].rearrange(
                        "(p g) c -> p (g c)", p=P
                    )
                    shin = SH[row0 : row0 + rows, :].rearrange(
                        "(p g) s -> p (g s)", p=P
                    )
                    oout = OUT[row0 : row0 + rows, :].rearrange(
                        "(p g) c -> p (g c)", p=P
                    )
                    sout = STK[row0 : row0 + rows, :].rearrange(
                        "(p g) one -> p (g one)", p=P
                    )

                    # Loads (all DMAs contiguous on both sides; strided
                    # moves happen on-chip)
                    xt = iopool.tile([P, G * DATA], F32, tag="xt")
                    nc.sync.dma_start(out=xt[:], in_=xin)
                    sh = iopool.tile([P, G * NSH], F32, tag="sh")
                    shv = sh[:].rearrange("p (g s) -> p g s", s=NSH)
                    nc.sync.dma_start(out=sh[:], in_=shin)

                    # copy_predicated wants an integer mask dtype
                    shu = iopool.tile([P, G * NSH], mybir.dt.uint8, tag="shu")
                    shuv = shu[:].rearrange("p (g s) -> p g s", s=NSH)
                    nc.vector.tensor_copy(out=shu[:], in_=sh[:])

                    xtv = xt[:].rearrange("p (g c) -> p g c", c=DATA)
                    nc.vector.tensor_copy(out=data, in_=xtv)

                    # Pre-shift row sums (for sticky)
                    sx = iopool.tile([P, G], F32, tag="sx")
                    nc.vector.tensor_reduce(
                        out=sx[:], in_=xtv, axis=mybir.AxisListType.X,
                        op=mybir.AluOpType.add,
                    )

                    # 6 blend layers, each one in-place predicated copy.
                    # Descending stream order (negative-step APs) makes the
                    # offset read safe in place.
                    for li, a in enumerate(SHIFT_AMTS):
                        dst = bufw[:, :, PAD:W][:, :, ::-1]
                        src = bufw[:, :, PAD - a : W - a][:, :, ::-1]
                        if G == 1:
                            mask = shu[0:P, li : li + 1].to_broadcast((P, DATA))
                        else:
                            mask = shuv[:, :, li : li + 1].to_broadcast((P, G, DATA))
                        nc.vector.copy_predicated(out=dst, mask=mask, data=src)

                    # Post-shift row sums -> sticky
                    so = iopool.tile([P, G], F32, tag="so")
                    nc.vector.tensor_reduce(
                        out=so[:], in_=data, axis=mybir.AxisListType.X,
                        op=mybir.AluOpType.add,
                    )
                    st = iopool.tile([P, G], F32, tag="st")
                    nc.vector.tensor_sub(st[:], sx[:], so[:])
                    nc.vector.tensor_scalar(
                        out=st[:], in0=st[:], scalar1=0.0, scalar2=None,
                        op0=mybir.AluOpType.is_gt,
                    )

                    # Contiguous staging for the output store
                    ot = iopool.tile([P, G * DATA], F32, tag="ot")
                    otv = ot[:].rearrange("p (g c) -> p g c", c=DATA)
                    nc.vector.tensor_copy(out=otv, in_=data)

                    # Stores
                    nc.sync.dma_start(out=oout, in_=ot[:])
                    nc.sync.dma_start(out=sout, in_=st[:])

                    row0 += rows
            assert row0 == R
    return nc


_CACHED = {}


def _get_compiled(R, plan):
    key = (R, tuple(plan))
    if key not in _CACHED:
        nc = bacc.Bacc(
            "TRN2",
            target_bir_lowering=False,
            debug=False,
            num_devices=N_CORES,
        )
        build_barrel(nc, R, plan)
        nc.compile()
        _CACHED[key] = nc
    return _CACHED[key]


def kernel(X: np.ndarray, shift: np.ndarray):
    assert X.shape == (B_FULL, DATA) and shift.shape == (B_FULL, NSH)
    X = np.ascontiguousarray(X, dtype=np.float32)
    shift = np.ascontiguousarray(shift, dtype=np.float32)

    nc = _get_compiled(R_CORE, FULL_PLAN)

    in_maps = []
    for c in range(N_CORES):
        sl = slice(c * R_CORE, (c + 1) * R_CORE)
        in_maps.append({"X": X[sl], "shift": shift[sl]})

    res = run_bass_kernel_spmd(nc, in_maps, list(range(N_CORES)))
    outs = np.concatenate([r["out"] for r in res.results], axis=0)
    sticky = np.concatenate([r["sticky"] for r in res.results], axis=0)
    return outs, sticky


# revision 11
# speedup vs baseline: 1.0743x; 1.0743x over previous
"""Trainium2 Bass kernel: 48-bit barrel shifter (right) with sticky bit.

Problem: X [B,48] of 0/1 f32 bits, shift [B,6] of 0/1 f32 bits (MSB first:
32,16,8,4,2,1). Output = (X >> S) with zero fill, plus sticky = OR of the
bits shifted out (= OR of last min(S,48) bits of X).

Strategy (pure data parallel over 8 NeuronCores, 125k rows/core):
  - Tile rows as [P=128 partitions, G rows/partition]; each row's 48 bits
    live in an 80-wide bf16 padded block (32 zeros | 48 data) so a right
    shift by `a` is a read at offset -a with zeros flowing in from the pad.
  - Bits are held as bf16; for the even shift layers (32,16,8,4,2) the
    blend runs on the buffer REINTERPRETED as fp32 pairs (halves the
    element count; copy_predicated is a 1x-mode op so elements = cycles).
  - Each layer is ONE in-place copy_predicated on the DVE, streamed in
    descending column order (negative-step APs) so the offset read always
    happens before the corresponding write. Mask = shift bit (cast to
    uint8) read through a stride-0 broadcast AP.
  - ScalarE does the fp32->bf16 in-cast (into the padded layout) and the
    bf16->fp32 out-cast; GPSIMD computes the pre-shift row sums via an
    add-tree; sticky = (sum_pre - sum_post) > 0 since bits are only moved
    or dropped.
"""

import numpy as np
from contextlib import ExitStack

import concourse.bacc as bacc
import concourse.mybir as mybir
from concourse import tile
from concourse.bass_utils import run_bass_kernel_spmd

DATA = 48
NSH = 6
PAD = 32
W = PAD + DATA  # 80 bf16 = 40 fp32 pairs
SHIFT_AMTS = [32, 16, 8, 4, 2, 1]

N_CORES = 8
B_FULL = 1_000_000
R_CORE = B_FULL // N_CORES  # 125000

# 125000 = 16 tiles of 128x61 (124928) + one tail tile of 72 rows.
FULL_PLAN = [(128, 61, 16), (72, 1, 1)]

F32 = mybir.dt.float32
BF16 = mybir.dt.bfloat16
U8 = mybir.dt.uint8


def build_barrel(nc, R, plan):
    X = nc.dram_tensor("X", [R, DATA], F32, kind="ExternalInput").ap()
    SH = nc.dram_tensor("shift", [R, NSH], F32, kind="ExternalInput").ap()
    OUT = nc.dram_tensor("out", [R, DATA], F32, kind="ExternalOutput").ap()
    STK = nc.dram_tensor("sticky", [R, 1], F32, kind="ExternalOutput").ap()

    g_max = max(g for (_, g, _) in plan)

    with tile.TileContext(nc) as tc:
        with ExitStack() as ctx:
            iopool = ctx.enter_context(tc.tile_pool(name="io", bufs=3))
            wpool = ctx.enter_context(tc.tile_pool(name="work", bufs=1))

            # Persistent ping-pong work buffers (bf16); pad columns stay
            # zero for the whole kernel (blends only write the data region).
            wbufs = [
                wpool.tile([128, g_max * W], BF16, tag=f"wb{i}", name=f"wb{i}")
                for i in range(2)
            ]
            for b in wbufs:
                nc.gpsimd.memset(b[:], 0.0)

            row0 = 0
            it = 0
            for (P, G, T) in plan:
                for _ in range(T):
                    rows = P * G
                    buf = wbufs[it % 2]
                    it += 1

                    # bf16 3D view and fp32-pair 3D view of the work buffer
                    bufb = buf[0:P, 0 : G * W].rearrange("p (g w) -> p g w", w=W)
                    bufp = (
                        buf[0:P, 0 : G * W]
                        .bitcast(F32)
                        .rearrange("p (g w) -> p g w", w=W // 2)
                    )
                    datab = bufb[:, :, PAD:W]  # [P, G, 48] bf16

                    xin = X[row0 : row0 + rows, :].rearrange(
                        "(p g) c -> p (g c)", p=P
                    )
                    shin = SH[row0 : row0 + rows, :].rearrange(
                        "(p g) s -> p (g s)", p=P
                    )
                    oout = OUT[row0 : row0 + rows, :].rearrange(
                        "(p g) c -> p (g c)", p=P
                    )
                    sout = STK[row0 : row0 + rows, :].rearrange(
                        "(p g) one -> p (g one)", p=P
                    )

                    # ---- loads (contiguous both sides) ----
                    xt = iopool.tile([P, G * DATA], F32, tag="xt")
                    nc.sync.dma_start(out=xt[:], in_=xin)
                    sh = iopool.tile([P, G * NSH], F32, tag="sh")
                    nc.sync.dma_start(out=sh[:], in_=shin)

                    # integer mask bits for copy_predicated
                    shu = iopool.tile([P, G * NSH], U8, tag="shu")
                    nc.vector.tensor_copy(out=shu[:], in_=sh[:])
                    shuv = shu[:].rearrange("p (g s) -> p g s", s=NSH)

                    # ---- in-cast fp32 -> bf16 into padded layout (ACT) ----
                    xtv = xt[:].rearrange("p (g c) -> p g c", c=DATA)
                    nc.scalar.copy(out=datab, in_=xtv)

                    # ---- pre-shift row sums on GPSIMD (add tree) ----
                    # widths 48->24->12->6->3->(2+1)->1, all fp32 out of xt
                    tr = iopool.tile([P, G * 24], F32, tag="tr")
                    trv = tr[:].rearrange("p (g c) -> p g c", c=24)
                    x3 = xtv
                    nc.gpsimd.tensor_add(
                        trv[:, :, 0:24], x3[:, :, 0:24], x3[:, :, 24:48]
                    )
                    nc.gpsimd.tensor_add(
                        trv[:, :, 0:12], trv[:, :, 0:12], trv[:, :, 12:24]
                    )
                    nc.gpsimd.tensor_add(
                        trv[:, :, 0:6], trv[:, :, 0:6], trv[:, :, 6:12]
                    )
                    nc.gpsimd.tensor_add(
                        trv[:, :, 0:3], trv[:, :, 0:3], trv[:, :, 3:6]
                    )
                    nc.gpsimd.tensor_add(
                        trv[:, :, 0:1], trv[:, :, 0:1], trv[:, :, 1:2]
                    )
                    sx = iopool.tile([P, G], F32, tag="sx")
                    sxv = sx[:].rearrange("p (g one) -> p g one", one=1)
                    nc.gpsimd.tensor_add(sxv, trv[:, :, 0:1], trv[:, :, 2:3])

                    # ---- 6 blend layers, in-place predicated copies ----
                    for li, a in enumerate(SHIFT_AMTS):
                        d = a // 2
                        wid = DATA // 2 if a > 1 else DATA
                        if G == 1:
                            if a > 1:
                                b2 = buf[0:P, 0:W].bitcast(F32)
                                dst = b2[:, PAD // 2 : W // 2][:, ::-1]
                                src = b2[:, PAD // 2 - d : W // 2 - d][:, ::-1]
                            else:
                                b2 = buf[0:P, 0:W]
                                dst = b2[:, PAD:W][:, ::-1]
                                src = b2[:, PAD - a : W - a][:, ::-1]
                            mask = shu[0:P, li : li + 1].to_broadcast((P, wid))
                        else:
                            if a > 1:
                                dst = bufp[:, :, PAD // 2 : W // 2][:, :, ::-1]
                                src = bufp[:, :, PAD // 2 - d : W // 2 - d][
                                    :, :, ::-1
                                ]
                            else:
                                dst = bufb[:, :, PAD:W][:, :, ::-1]
                                src = bufb[:, :, PAD - a : W - a][:, :, ::-1]
                            mask = shuv[:, :, li : li + 1].to_broadcast((P, G, wid))
                        nc.vector.copy_predicated(out=dst, mask=mask, data=src)

                    # ---- post-shift row sums on DVE (bf16 add tree) ----
                    tb = iopool.tile([P, G * 24], BF16, tag="tb")
                    tbv = tb[:].rearrange("p (g c) -> p g c", c=24)
                    nc.vector.tensor_add(
                        tbv[:, :, 0:24], datab[:, :, 0:24], datab[:, :, 24:48]
                    )
                    nc.vector.tensor_add(
                        tbv[:, :, 0:12], tbv[:, :, 0:12], tbv[:, :, 12:24]
                    )
                    nc.vector.tensor_add(
                        tbv[:, :, 0:6], tbv[:, :, 0:6], tbv[:, :, 6:12]
                    )
                    so = iopool.tile([P, G], F32, tag="so")
                    nc.vector.tensor_reduce(
                        out=so[:],
                        in_=tbv[:, :, 0:6],
                        axis=mybir.AxisListType.X,
                        op=mybir.AluOpType.add,
                    )

                    # sticky = (sum_pre - sum_post) > 0   (GPSIMD smalls)
                    st = iopool.tile([P, G], F32, tag="st")
                    nc.gpsimd.tensor_sub(st[:], sx[:], so[:])
                    nc.gpsimd.tensor_scalar(
                        out=st[:], in0=st[:], scalar1=0.0, scalar2=None,
                        op0=mybir.AluOpType.is_gt,
                    )

                    # ---- out-cast bf16 -> fp32 (ACT) + stores ----
                    ot = iopool.tile([P, G * DATA], F32, tag="ot")
                    otv = ot[:].rearrange("p (g c) -> p g c", c=DATA)
                    nc.scalar.copy(out=otv, in_=datab)

                    nc.sync.dma_start(out=oout, in_=ot[:])
                    nc.sync.dma_start(out=sout, in_=st[:])

                    row0 += rows
            assert row0 == R
    return nc


_CACHED = {}


def _get_compiled(R, plan):
    key = (R, tuple(plan))
    if key not in _CACHED:
        nc = bacc.Bacc(
            "TRN2",
            target_bir_lowering=False,
            debug=False,
            num_devices=N_CORES,
        )
        build_barrel(nc, R, plan)
        nc.compile()
        _CACHED[key] = nc
    return _CACHED[key]


def kernel(X: np.ndarray, shift: np.ndarray):
    assert X.shape == (B_FULL, DATA) and shift.shape == (B_FULL, NSH)
    X = np.ascontiguousarray(X, dtype=np.float32)
    shift = np.ascontiguousarray(shift, dtype=np.float32)

    nc = _get_compiled(R_CORE, FULL_PLAN)

    in_maps = []
    for c in range(N_CORES):
        sl = slice(c * R_CORE, (c + 1) * R_CORE)
        in_maps.append({"X": X[sl], "shift": shift[sl]})

    res = run_bass_kernel_spmd(nc, in_maps, list(range(N_CORES)))
    outs = np.concatenate([r["out"] for r in res.results], axis=0)
    sticky = np.concatenate([r["sticky"] for r in res.results], axis=0)
    return outs, sticky
